# revision 14
# baseline (speedup 1.0000x reference)
"""Distributed Trainium2 kernel for nn_AttentionLayer (B=2, S=2048, D=2048, H=16).

Sharding: core c = (batch b, head-group g) with b = c // 4, g = c % 4.
Each core owns 4 heads (512 of the 2048 projection dims) of one batch element:
projections (bf16 matmuls, f32 accumulation), masked softmax attention for its
4 heads (no max-subtraction; masked entries become 0 via exp(s)*(1-mask)),
and its partial output projection (Wo row-shard). The 4 partial outputs per
batch are summed on the host (cross-core collectives hang on the axon PJRT
path in this container).

v2 structure:
- Phase A m-outer / k-contiguous accumulation: each (npair, m) runs its full
  16-chunk contraction back-to-back into one [128,1024] PSUM tile (2 banks),
  so evictions overlap the next m's matmuls and the PE never waits on PSUM.
- Activations stream as [128,1024] DMAs (sync queue); weights arrive as four
  [128,2048] DMAs per matrix in host-packed layout (gpsimd queue) so the
  first matmul starts ~2us in. All output DMAs also ride the gpsimd queue.
- Phase B processes scores in [128,1024] skc-pairs: one exp (ScalarE) and one
  mask-mul (DVE, 2x 16-bit mode) per pair, halving elementwise instruction
  count. Emission interleaves scores(i+1) pairs with ctx(i) matmul chunks so
  PSUM stays shallow (2 score tiles) and the PE stays dense.
- ctx keeps the ones-column trick (vpo 129-wide per head) for the softmax
  denominator; the 128x128 PE transposes of normalized ctx are deferred one
  full step (their DVE inputs are long since ready) and batched into a single
  [128,512] bf16 PSUM tile followed by one 2x DVE copy.
- Out-projection PSUM evictions split between ScalarE and DVE.
"""

import numpy as np
import ml_dtypes

import concourse.bass as bass  # noqa: F401
import concourse.mybir as mybir
import concourse.tile as tile
from concourse import bacc
from concourse import bass_utils
from concourse.masks import make_identity

BF16 = mybir.dt.bfloat16
F32 = mybir.dt.float32
nbf16 = ml_dtypes.bfloat16

B, S, D, H = 2, 2048, 2048, 16
GH = 4                # heads per core
DH = 128              # head dim
GD = GH * DH          # 512 local projection dims
KC = D // 128         # 16 contraction chunks
NB = 4                # query blocks
BLK = S // NB         # 512
NJ = KC // 2          # 8 skc-pairs per step
N_CORES = 8
SCALE = float(1.0 / np.sqrt(DH))

_CACHE = {}


def _build():
    nc = bacc.Bacc(
        "TRN2", target_bir_lowering=False, debug=False, num_devices=N_CORES
    )
    AF = mybir.ActivationFunctionType

    qT = nc.dram_tensor("qT", [D, S], BF16, kind="ExternalInput")
    kT = nc.dram_tensor("kT", [D, S], BF16, kind="ExternalInput")
    vT = nc.dram_tensor("vT", [D, S], BF16, kind="ExternalInput")
    maskP = nc.dram_tensor("maskP", [S // 2, 2 * S], BF16, kind="ExternalInput")
    wqP = nc.dram_tensor("wqP", [128, KC * GD], BF16, kind="ExternalInput")
    wkP = nc.dram_tensor("wkP", [128, KC * GD], BF16, kind="ExternalInput")
    wvP = nc.dram_tensor("wvP", [128, KC * GD], BF16, kind="ExternalInput")
    woP = nc.dram_tensor("woP", [128, GH * D], BF16, kind="ExternalInput")
    kpT_out = nc.dram_tensor("kpT_out", [GD, S], F32, kind="ExternalOutput")
    vp_out = nc.dram_tensor("vp_out", [S, GD], F32, kind="ExternalOutput")
    outp_out = nc.dram_tensor("outp_out", [S, D], BF16, kind="ExternalOutput")

    with tile.TileContext(nc) as tc:
        with tc.tile_pool(name="res", bufs=1) as res:
            # ---- resident SBUF tensors (live across both phases) ----
            wo_sb = res.tile([128, GH * D], BF16, name="wo_sb", tag="wo")
            qp_sb = res.tile([128, GH * S], BF16, name="qp_sb", tag="qp")
            kp_sb = res.tile([128, GH * S], BF16, name="kp_sb", tag="kp")
            # vp + per-head ones column: s-chunk sc at cols sc*516, head h at
            # +h*129 (128 vp dims then one 1.0 column for the softmax denom)
            vpo_sb = res.tile([128, KC * 516], BF16, name="vpo_sb", tag="vpo")
            ident = res.tile([128, 128], BF16, name="ident", tag="ident")

            # ---------------- Phase A: projections ----------------
            with (
                tc.tile_pool(name="wpool", bufs=1) as wpool,
                tc.tile_pool(name="stream", bufs=18) as stream,
                tc.tile_pool(name="stageA", bufs=4) as stageA,
                tc.tile_pool(name="psA", bufs=4, space="PSUM") as psA,
            ):
                wq_sb = wpool.tile([128, KC * GD], BF16, name="wq_sb", tag="wq")
                wk_sb = wpool.tile([128, KC * GD], BF16, name="wk_sb", tag="wk")
                wv_sb = wpool.tile([128, KC * GD], BF16, name="wv_sb", tag="wv")

                # Weight DMAs ride the gpsimd queue.  The framework coalesces
                # DMA waits per queue (a consumer waits for everything emitted
                # so far on that queue), so weights are staged just-in-time:
                # wv up front, wk/wq/wo during the groups that precede their
                # first use (see weight_stage below).
                def emit_weight(w_sb_, wP_):
                    for qtr in range(4):
                        nc.gpsimd.dma_start(
                            out=w_sb_[:, qtr * 2048:(qtr + 1) * 2048],
                            in_=wP_[:, qtr * 2048:(qtr + 1) * 2048],
                        )

                emit_weight(wv_sb, wvP)
                make_identity(nc, ident[:])
                nc.gpsimd.memset(vpo_sb[:], 1.0)

                # Projection groups, order vp -> kp -> qp so the final
                # evictions before phase B are cheap (qp: one ACT copy) and
                # the kp/vp output DMAs drain early.  Each group is one
                # (matrix, npair): 16 [128,1024] activation tiles feeding
                # 4 m-chains that accumulate k-contiguously in [128,1024]
                # PSUM tiles.  Activation DMAs for group g+1 are issued
                # during group g's 3rd/4th m-chains (prefetch), so chains
                # never outrun the DMA queue; the very first group instead
                # runs kc-outer (8 matmuls per tile) to tolerate the cold
                # DMA stream.
                groups = []
                for mqp in range(2):
                    groups.append(("v", mqp))
                for xk, w_sb_, dst in (("k", wk_sb, kp_sb), ("q", wq_sb, qp_sb)):
                    for npair in range(2):
                        groups.append((xk, npair))
                xdram_of = {"v": vT, "k": kT, "q": qT}

                def issue_xs_dmas(gi, half):
                    xk, npair = groups[gi]
                    xd = xdram_of[xk]
                    xss = xss_of.setdefault(gi, [])
                    for kc in range(half * 8, half * 8 + 8):
                        xs = stream.tile([128, 1024], BF16, name="xs", tag="xs")
                        eng = nc.sync if kc % 2 == 0 else nc.gpsimd
                        eng.dma_start(
                            out=xs[:],
                            in_=xd[kc * 128:(kc + 1) * 128,
                                   npair * 1024:(npair + 1) * 1024],
                        )
                        xss.append(xs)

                def emit_qk_chain(w_sb_, dst_bf, f32out, npair, m, xss):
                    ps = psA.tile([128, 1024], F32, name="ps", tag="psa")
                    for kc in range(KC):
                        for h2 in range(2):
                            nc.tensor.matmul(
                                ps[:, h2 * 512:(h2 + 1) * 512],
                                lhsT=w_sb_[:, kc * GD + m * 128:
                                           kc * GD + (m + 1) * 128],
                                rhs=xss[kc][:, h2 * 512:(h2 + 1) * 512],
                                start=(kc == 0),
                                stop=(kc == KC - 1),
                            )
                    dst = dst_bf[:, m * S + npair * 1024:
                                 m * S + (npair + 1) * 1024]
                    # alternate eviction engines so the next group's psum
                    # reuse never waits on this group's last eviction
                    if m % 2 == 0:
                        nc.scalar.copy(dst, ps[:])
                    else:
                        nc.vector.tensor_copy(dst, ps[:])
                    if f32out is not None:
                        st = stageA.tile([128, 1024], F32, name="st", tag="st")
                        if m % 2 == 0:
                            nc.vector.tensor_copy(st[:], ps[:])
                        else:
                            nc.scalar.copy(st[:], ps[:])
                        nc.scalar.dma_start(
                            out=f32out[m * 128:(m + 1) * 128,
                                       npair * 1024:(npair + 1) * 1024],
                            in_=st[:],
                        )

                def emit_v_evict(ps, mqp, sp):
                    for h2 in range(2):
                        sc = mqp * 8 + sp * 2 + h2
                        for h in range(GH):
                            dst = vpo_sb[:, sc * 516 + h * 129:
                                         sc * 516 + h * 129 + 128]
                            src = ps[:, h2 * 512 + h * 128:
                                     h2 * 512 + (h + 1) * 128]
                            if sp % 2 == 0:
                                nc.scalar.copy(dst, src)
                            else:
                                nc.vector.tensor_copy(dst, src)
                    st = stageA.tile([128, 1024], F32, name="st", tag="st")
                    if sp % 2 == 0:
                        nc.vector.tensor_copy(st[:], ps[:])
                    else:
                        nc.scalar.copy(st[:], ps[:])
                    for h2 in range(2):
                        sc = mqp * 8 + sp * 2 + h2
                        nc.scalar.dma_start(
                            out=vp_out[sc * 128:(sc + 1) * 128, :],
                            in_=st[:, h2 * 512:(h2 + 1) * 512],
                        )

                def emit_v_chain(mqp, sp, xss):
                    ps = psA.tile([128, 1024], F32, name="ps", tag="psa")
                    for kc in range(KC):
                        for h2 in range(2):
                            sl = sp * 2 + h2
                            nc.tensor.matmul(
                                ps[:, h2 * 512:(h2 + 1) * 512],
                                lhsT=xss[kc][:, sl * 128:(sl + 1) * 128],
                                rhs=wv_sb[:, kc * GD:(kc + 1) * GD],
                                start=(kc == 0),
                                stop=(kc == KC - 1),
                            )
                    emit_v_evict(ps, mqp, sp)

                xss_of = {}
                issue_xs_dmas(0, 0)
                issue_xs_dmas(0, 1)
                # weights staged just-in-time: emit during the group that
                # precedes first use (wk before k groups, wq before q, wo
                # for phase B)
                weight_stage = {1: (wk_sb, wkP), 3: (wq_sb, wqP),
                                5: (wo_sb, woP)}
                for gi, (xk, npair) in enumerate(groups):
                    xss = xss_of[gi]
                    if gi in weight_stage:
                        emit_weight(*weight_stage[gi])
                    if gi == 0:
                        # kc-outer: 8 matmuls per tile, tolerant of the
                        # cold DMA stream; 4 PSUM tiles live through it.
                        pss = [psA.tile([128, 1024], F32, name="ps", tag="psa")
                               for _ in range(4)]
                        for kc in range(KC):
                            for sp in range(4):
                                for h2 in range(2):
                                    sl = sp * 2 + h2
                                    nc.tensor.matmul(
                                        pss[sp][:, h2 * 512:(h2 + 1) * 512],
                                        lhsT=xss[kc][:, sl * 128:(sl + 1) * 128],
                                        rhs=wv_sb[:, kc * GD:(kc + 1) * GD],
                                        start=(kc == 0),
                                        stop=(kc == KC - 1),
                                    )
                        issue_xs_dmas(1, 0)
                        issue_xs_dmas(1, 1)
                        for sp in range(4):
                            emit_v_evict(pss[sp], 0, sp)
                        continue
                    for m in range(4):
                        if m == 2 and gi + 1 < len(groups):
                            issue_xs_dmas(gi + 1, 0)
                        if m == 3 and gi + 1 < len(groups):
                            issue_xs_dmas(gi + 1, 1)
                        if xk == "v":
                            emit_v_chain(npair, m, xss)
                        elif xk == "k":
                            emit_qk_chain(wk_sb, kp_sb, kpT_out, npair, m, xss)
                        else:
                            emit_qk_chain(wq_sb, qp_sb, None, npair, m, xss)

            # ---------------- Phase B: attention + out-proj ----------------
            with (
                tc.tile_pool(name="mp", bufs=16) as mp,
                tc.tile_pool(name="apl", bufs=20) as apl,
                tc.tile_pool(name="cpl", bufs=8) as cpl,
                tc.tile_pool(name="stageB", bufs=12) as stageB,
                tc.tile_pool(name="ostp", bufs=6) as ostp,
                tc.tile_pool(name="psS", bufs=2, space="PSUM") as psS,
                tc.tile_pool(name="psCT", bufs=2, space="PSUM") as psCT,
                tc.tile_pool(name="psOT", bufs=2, space="PSUM") as psOT,
            ):
                steps = [(t, h) for t in range(NB) for h in range(GH)]
                mask_tiles = {}    # t -> list of 8 pair tiles
                at_tiles = {}      # (i, j) -> at pair tile
                cps_tiles = {}     # (i, jj//2) -> cps tile holding 2 chunks
                ctn_tiles = {}     # (i, jj) -> normalized ctx [128,128]
                cth_tiles = {}     # (t, h) -> [128,512] transposed ctx

                def emit_mask_dmas(t):
                    mts = []
                    for j in range(NJ):
                        mt = mp.tile([128, 1024], BF16, name="mt", tag="mt")
                        nc.sync.dma_start(
                            out=mt[:],
                            in_=maskP[j * 128:(j + 1) * 128,
                                      t * 1024:(t + 1) * 1024],
                        )
                        mts.append(mt)
                    mask_tiles[t] = mts

                def emit_scores_pair(i, j):
                    t, h = steps[i]
                    sps = psS.tile([128, 1024], F32, name="sps", tag="sps")
                    for c in range(2):
                        skc = 2 * j + c
                        nc.tensor.matmul(
                            sps[:, c * 512:(c + 1) * 512],
                            lhsT=kp_sb[:, h * S + skc * 128:
                                       h * S + (skc + 1) * 128],
                            rhs=qp_sb[:, h * S + t * 512: h * S + (t + 1) * 512],
                            start=True,
                            stop=True,
                        )
                    at = apl.tile([128, 1024], BF16, name="at", tag="at")
                    nc.scalar.activation(at[:], sps[:], AF.Exp, scale=SCALE)
                    nc.vector.tensor_mul(at[:], at[:], mask_tiles[t][j][:])
                    at_tiles[(i, j)] = at

                def emit_ctx_part(i, j):
                    # ctx matmuls for step i, slice j: mm-chunk jj = j//2,
                    # skc range (j%2)*8 .. +8; finalize (recip+norm) at odd j.
                    t, h = steps[i]
                    jj = j // 2
                    if j % 2 == 0 and jj % 2 == 0:
                        cps_tiles[(i, jj // 2)] = psCT.tile(
                            [128, 512], F32, name="cps", tag="cps")
                    cps = cps_tiles[(i, jj // 2)]
                    off = (jj % 2) * 129
                    for skc in range((j % 2) * 8, (j % 2) * 8 + 8):
                        at = at_tiles[(i, skc // 2)]
                        nc.tensor.matmul(
                            cps[:, off:off + 129],
                            lhsT=at[:, (skc % 2) * 512 + jj * 128:
                                    (skc % 2) * 512 + (jj + 1) * 128],
                            rhs=vpo_sb[:, skc * 516 + h * 129:
                                       skc * 516 + (h + 1) * 129],
                            start=(skc == 0),
                            stop=(skc == KC - 1),
                        )
                    if j % 2 == 1:
                        rec = stageB.tile([128, 1], F32, name="rec", tag="rec")
                        nc.vector.reciprocal(rec[:], cps[:, off + 128:off + 129])
                        ctn = stageB.tile([128, 128], BF16, name="ctn",
                                          tag="ctn")
                        nc.vector.tensor_scalar_mul(
                            ctn[:], cps[:, off:off + 128], rec[:])
                        ctn_tiles[(i, jj)] = ctn
                    if j == 7:
                        # release the at tiles of step i
                        for jd in range(NJ):
                            at_tiles.pop((i, jd), None)

                def emit_transposes(i):
                    t, h = steps[i]
                    tps = psOT.tile([128, 512], BF16, name="tps", tag="psot")
                    for mm in range(4):
                        nc.tensor.transpose(
                            tps[:, mm * 128:(mm + 1) * 128],
                            ctn_tiles.pop((i, mm))[:], ident[:])
                    cth = cpl.tile([128, 512], BF16, name="cth", tag="cth")
                    nc.vector.tensor_copy(cth[:], tps[:])
                    cth_tiles[(t, h)] = cth

                def emit_outproj(t):
                    cts = [cth_tiles.pop((t, h)) for h in range(GH)]
                    for mm in range(4):
                        for npair in range(2):
                            ops2 = [psOT.tile([128, 512], F32, name=f"ops{j2}",
                                              tag="psot") for j2 in range(2)]
                            for h in range(GH):
                                for n2 in range(2):
                                    n = npair * 2 + n2
                                    nc.tensor.matmul(
                                        ops2[n2][:],
                                        lhsT=cts[h][:, mm * 128:(mm + 1) * 128],
                                        rhs=wo_sb[:, h * D + n * 512:
                                                  h * D + (n + 1) * 512],
                                        start=(h == 0),
                                        stop=(h == GH - 1),
                                    )
                            ost = ostp.tile([128, 1024], BF16, name="ost",
                                            tag="ost")
                            nc.scalar.copy(ost[:, 0:512], ops2[0][:])
                            nc.vector.tensor_copy(ost[:, 512:1024], ops2[1][:])
                            eng = nc.sync if npair == 0 else nc.gpsimd
                            eng.dma_start(
                                out=outp_out[t * BLK + mm * 128:
                                             t * BLK + (mm + 1) * 128,
                                             npair * 1024:(npair + 1) * 1024],
                                in_=ost[:],
                            )

                # -------- software pipeline --------
                emit_mask_dmas(0)
                for i in range(len(steps) + 1):
                    t, h = steps[i] if i < len(steps) else (None, None)
                    for j in range(NJ):
                        if i < len(steps):
                            emit_scores_pair(i, j)
                        if j == 1 and i >= 2:
                            emit_transposes(i - 2)
                        if j == 3 and i < len(steps) and h == 2:
                            emit_mask_dmas(t + 1) if t + 1 < NB else None
                        if i >= 1:
                            emit_ctx_part(i - 1, j)
                    if i >= 5 and steps[i - 1][1] == 0:
                        # transposes of block t'-1 all done (last at i-2, j=1)
                        emit_outproj(steps[i - 2][0])
                # epilogue: transposes of the last two steps, final outproj
                emit_transposes(len(steps) - 1)
                emit_outproj(NB - 1)

    nc.compile()
    return nc


def get_nc():
    if "nc" not in _CACHE:
        _CACHE["nc"] = _build()
    return _CACHE["nc"]


def make_in_maps(inputs):
    q = np.asarray(inputs["q"], np.float32)
    k = np.asarray(inputs["k"], np.float32)
    v = np.asarray(inputs["v"], np.float32)
    mask = np.asarray(inputs["mask"])
    Wq = np.asarray(inputs["Wq"], np.float32)
    Wk = np.asarray(inputs["Wk"], np.float32)
    Wv = np.asarray(inputs["Wv"], np.float32)
    Wo = np.asarray(inputs["Wo"], np.float32)

    per_batch = []
    for b in range(B):
        maskTb = np.ascontiguousarray(
            (~mask[b].astype(bool)).T).astype(nbf16)  # [key, q]
        # pair-packed mask: row j*128+p, col t*1024 + c*512 + g
        #   = maskTb[j*256 + c*128 + p, t*512 + g]
        maskP = np.ascontiguousarray(
            maskTb.reshape(NJ, 2, 128, NB, 512)
            .transpose(0, 2, 3, 1, 4).reshape(S // 2, 2 * S))
        per_batch.append({
            "qT": np.ascontiguousarray(q[b].T).astype(nbf16),
            "kT": np.ascontiguousarray(k[b].T).astype(nbf16),
            "vT": np.ascontiguousarray(v[b].T).astype(nbf16),
            "maskP": maskP,
        })

    def packw(wT, ncols):
        # wT [rows, ncols] -> [128, (rows//128)*ncols] with chunk-major cols
        r = wT.shape[0] // 128
        return np.ascontiguousarray(
            wT.reshape(r, 128, ncols).transpose(1, 0, 2).reshape(128, r * ncols))

    per_group = []
    for g in range(4):
        sl = slice(g * GD, (g + 1) * GD)
        per_group.append({
            "wqP": packw(np.ascontiguousarray(Wq[sl, :].T).astype(nbf16), GD),
            "wkP": packw(np.ascontiguousarray(Wk[sl, :].T).astype(nbf16), GD),
            "wvP": packw(np.ascontiguousarray(Wv[sl, :].T).astype(nbf16), GD),
            "woP": packw(np.ascontiguousarray(Wo[:, sl].T).astype(nbf16), D),
        })
    in_maps = []
    for c in range(N_CORES):
        b, g = c // 4, c % 4
        m = {}
        m.update(per_batch[b])
        m.update(per_group[g])
        in_maps.append(m)
    return in_maps


def assemble(results):
    out = np.zeros((B, S, D), np.float32)
    kp = np.empty((B, S, D), np.float32)
    vp = np.empty((B, S, D), np.float32)
    for c, res in enumerate(results):
        b, g = c // 4, c % 4
        kp[b][:, g * GD:(g + 1) * GD] = res["kpT_out"].T
        vp[b][:, g * GD:(g + 1) * GD] = res["vp_out"]
        out[b] += res["outp_out"].astype(np.float32)
    return out, kp, vp


def run_cores(in_maps, trace=False, **kwargs):
    nc = get_nc()
    return bass_utils.run_bass_kernel_spmd(
        nc, in_maps, core_ids=list(range(N_CORES)), trace=trace, **kwargs
    )


def kernel(**inputs):
    in_maps = make_in_maps(inputs)
    res = run_cores(in_maps, trace=False)
    return assemble(res.results)


# revision 15
# speedup vs baseline: 1.0541x; 1.0541x over previous
"""Distributed Trainium2 kernel for nn_AttentionLayer (B=2, S=2048, D=2048, H=16).

Sharding: core c = (batch b, head-group g) with b = c // 4, g = c % 4.
Each core owns 4 heads (512 of the 2048 projection dims) of one batch element:
projections (bf16 matmuls, f32 accumulation), masked softmax attention for its
4 heads (no max-subtraction; masked entries become 0 via exp(s)*(1-mask)),
and its partial output projection (Wo row-shard). The 4 partial outputs per
batch are summed on the host (cross-core collectives hang on the axon PJRT
path in this container).

v2 structure:
- Phase A m-outer / k-contiguous accumulation: each (npair, m) runs its full
  16-chunk contraction back-to-back into one [128,1024] PSUM tile (2 banks),
  so evictions overlap the next m's matmuls and the PE never waits on PSUM.
- Activations stream as [128,1024] DMAs (sync queue); weights arrive as four
  [128,2048] DMAs per matrix in host-packed layout (gpsimd queue) so the
  first matmul starts ~2us in. All output DMAs also ride the gpsimd queue.
- Phase B processes scores in [128,1024] skc-pairs: one exp (ScalarE) and one
  mask-mul (DVE, 2x 16-bit mode) per pair, halving elementwise instruction
  count. Emission interleaves scores(i+1) pairs with ctx(i) matmul chunks so
  PSUM stays shallow (2 score tiles) and the PE stays dense.
- ctx keeps the ones-column trick (vpo 129-wide per head) for the softmax
  denominator; the 128x128 PE transposes of normalized ctx are deferred one
  full step (their DVE inputs are long since ready) and batched into a single
  [128,512] bf16 PSUM tile followed by one 2x DVE copy.
- Out-projection PSUM evictions split between ScalarE and DVE.
"""

import numpy as np
import ml_dtypes

import concourse.bass as bass  # noqa: F401
import concourse.mybir as mybir
import concourse.tile as tile
from concourse import bacc
from concourse import bass_utils
from concourse.masks import make_identity

BF16 = mybir.dt.bfloat16
F32 = mybir.dt.float32
nbf16 = ml_dtypes.bfloat16

B, S, D, H = 2, 2048, 2048, 16
GH = 4                # heads per core
DH = 128              # head dim
GD = GH * DH          # 512 local projection dims
KC = D // 128         # 16 contraction chunks
NB = 4                # query blocks
BLK = S // NB         # 512
NJ = KC // 2          # 8 skc-pairs per step
N_CORES = 8
SCALE = float(1.0 / np.sqrt(DH))

_CACHE = {}


def _build():
    nc = bacc.Bacc(
        "TRN2", target_bir_lowering=False, debug=False, num_devices=N_CORES
    )
    AF = mybir.ActivationFunctionType

    qT = nc.dram_tensor("qT", [D, S], BF16, kind="ExternalInput")
    kT = nc.dram_tensor("kT", [D, S], BF16, kind="ExternalInput")
    vT = nc.dram_tensor("vT", [D, S], BF16, kind="ExternalInput")
    maskP = nc.dram_tensor("maskP", [S // 2, 2 * S], BF16, kind="ExternalInput")
    wqP = nc.dram_tensor("wqP", [128, KC * GD], BF16, kind="ExternalInput")
    wkP = nc.dram_tensor("wkP", [128, KC * GD], BF16, kind="ExternalInput")
    wvP = nc.dram_tensor("wvP", [128, KC * GD], BF16, kind="ExternalInput")
    woP = nc.dram_tensor("woP", [128, GH * D], BF16, kind="ExternalInput")
    kpT_out = nc.dram_tensor("kpT_out", [GD, S], F32, kind="ExternalOutput")
    vp_out = nc.dram_tensor("vp_out", [S, GD], F32, kind="ExternalOutput")
    outp_out = nc.dram_tensor("outp_out", [S, D], BF16, kind="ExternalOutput")

    with tile.TileContext(nc) as tc:
        with tc.tile_pool(name="res", bufs=1) as res:
            # ---- resident SBUF tensors (live across both phases) ----
            wo_sb = res.tile([128, GH * D], BF16, name="wo_sb", tag="wo")
            qp_sb = res.tile([128, GH * S], BF16, name="qp_sb", tag="qp")
            kp_sb = res.tile([128, GH * S], BF16, name="kp_sb", tag="kp")
            # vp + per-head ones column: s-chunk sc at cols sc*516, head h at
            # +h*129 (128 vp dims then one 1.0 column for the softmax denom)
            vpo_sb = res.tile([128, KC * 516], BF16, name="vpo_sb", tag="vpo")
            ident = res.tile([128, 128], BF16, name="ident", tag="ident")

            # ---------------- Phase A: projections ----------------
            with (
                tc.tile_pool(name="wpool", bufs=1) as wpool,
                tc.tile_pool(name="stream", bufs=18) as stream,
                tc.tile_pool(name="stageA", bufs=4) as stageA,
                tc.tile_pool(name="psA", bufs=4, space="PSUM") as psA,
            ):
                wq_sb = wpool.tile([128, KC * GD], BF16, name="wq_sb", tag="wq")
                wk_sb = wpool.tile([128, KC * GD], BF16, name="wk_sb", tag="wk")
                wv_sb = wpool.tile([128, KC * GD], BF16, name="wv_sb", tag="wv")

                # Weight DMAs ride the gpsimd queue.  The framework coalesces
                # DMA waits per queue (a consumer waits for everything emitted
                # so far on that queue), so weights are staged just-in-time:
                # wv up front, wk/wq/wo during the groups that precede their
                # first use (see weight_stage below).
                def emit_weight(w_sb_, wP_):
                    for qtr in range(4):
                        nc.gpsimd.dma_start(
                            out=w_sb_[:, qtr * 2048:(qtr + 1) * 2048],
                            in_=wP_[:, qtr * 2048:(qtr + 1) * 2048],
                        )

                emit_weight(wv_sb, wvP)
                make_identity(nc, ident[:])
                nc.gpsimd.memset(vpo_sb[:], 1.0)

                # Projection groups, order vp -> kp -> qp so the final
                # evictions before phase B are cheap (qp: one ACT copy) and
                # the kp/vp output DMAs drain early.  Each group is one
                # (matrix, npair): 16 [128,1024] activation tiles feeding
                # 4 m-chains that accumulate k-contiguously in [128,1024]
                # PSUM tiles.  Activation DMAs for group g+1 are issued
                # during group g's 3rd/4th m-chains (prefetch), so chains
                # never outrun the DMA queue; the very first group instead
                # runs kc-outer (8 matmuls per tile) to tolerate the cold
                # DMA stream.
                groups = []
                for mqp in range(2):
                    groups.append(("v", mqp))
                for xk, w_sb_, dst in (("k", wk_sb, kp_sb), ("q", wq_sb, qp_sb)):
                    for npair in range(2):
                        groups.append((xk, npair))
                xdram_of = {"v": vT, "k": kT, "q": qT}

                def issue_xs_dmas(gi, half):
                    xk, npair = groups[gi]
                    xd = xdram_of[xk]
                    xss = xss_of.setdefault(gi, [])
                    for kc in range(half * 8, half * 8 + 8):
                        xs = stream.tile([128, 1024], BF16, name="xs", tag="xs")
                        eng = nc.sync if kc % 2 == 0 else nc.scalar
                        eng.dma_start(
                            out=xs[:],
                            in_=xd[kc * 128:(kc + 1) * 128,
                                   npair * 1024:(npair + 1) * 1024],
                        )
                        xss.append(xs)

                def emit_qk_chain(w_sb_, dst_bf, f32out, npair, m, xss):
                    ps = psA.tile([128, 1024], F32, name="ps", tag="psa")
                    for kc in range(KC):
                        for h2 in range(2):
                            nc.tensor.matmul(
                                ps[:, h2 * 512:(h2 + 1) * 512],
                                lhsT=w_sb_[:, kc * GD + m * 128:
                                           kc * GD + (m + 1) * 128],
                                rhs=xss[kc][:, h2 * 512:(h2 + 1) * 512],
                                start=(kc == 0),
                                stop=(kc == KC - 1),
                            )
                    dst = dst_bf[:, m * S + npair * 1024:
                                 m * S + (npair + 1) * 1024]
                    # alternate eviction engines so the next group's psum
                    # reuse never waits on this group's last eviction
                    if m % 2 == 0:
                        nc.scalar.copy(dst, ps[:])
                    else:
                        nc.vector.tensor_copy(dst, ps[:])
                    if f32out is not None:
                        st = stageA.tile([128, 1024], F32, name="st", tag="st")
                        if m % 2 == 0:
                            nc.vector.tensor_copy(st[:], ps[:])
                        else:
                            nc.scalar.copy(st[:], ps[:])
                        nc.scalar.dma_start(
                            out=f32out[m * 128:(m + 1) * 128,
                                       npair * 1024:(npair + 1) * 1024],
                            in_=st[:],
                        )

                def emit_v_evict(ps, mqp, sp):
                    for h2 in range(2):
                        sc = mqp * 8 + sp * 2 + h2
                        for h in range(GH):
                            dst = vpo_sb[:, sc * 516 + h * 129:
                                         sc * 516 + h * 129 + 128]
                            src = ps[:, h2 * 512 + h * 128:
                                     h2 * 512 + (h + 1) * 128]
                            if sp % 2 == 0:
                                nc.scalar.copy(dst, src)
                            else:
                                nc.vector.tensor_copy(dst, src)
                    st = stageA.tile([128, 1024], F32, name="st", tag="st")
                    if sp % 2 == 0:
                        nc.vector.tensor_copy(st[:], ps[:])
                    else:
                        nc.scalar.copy(st[:], ps[:])
                    for h2 in range(2):
                        sc = mqp * 8 + sp * 2 + h2
                        nc.scalar.dma_start(
                            out=vp_out[sc * 128:(sc + 1) * 128, :],
                            in_=st[:, h2 * 512:(h2 + 1) * 512],
                        )

                def emit_v_chain(mqp, sp, xss):
                    ps = psA.tile([128, 1024], F32, name="ps", tag="psa")
                    for kc in range(KC):
                        for h2 in range(2):
                            sl = sp * 2 + h2
                            nc.tensor.matmul(
                                ps[:, h2 * 512:(h2 + 1) * 512],
                                lhsT=xss[kc][:, sl * 128:(sl + 1) * 128],
                                rhs=wv_sb[:, kc * GD:(kc + 1) * GD],
                                start=(kc == 0),
                                stop=(kc == KC - 1),
                            )
                    emit_v_evict(ps, mqp, sp)

                xss_of = {}
                issue_xs_dmas(0, 0)
                issue_xs_dmas(0, 1)
                # weights staged just-in-time: emit during the group that
                # precedes first use (wk before k groups, wq before q, wo
                # for phase B)
                weight_stage = {1: (wk_sb, wkP), 3: (wq_sb, wqP),
                                5: (wo_sb, woP)}
                for gi, (xk, npair) in enumerate(groups):
                    xss = xss_of[gi]
                    if gi in weight_stage:
                        emit_weight(*weight_stage[gi])
                    if gi == 0:
                        # kc-outer: 8 matmuls per tile, tolerant of the
                        # cold DMA stream; 4 PSUM tiles live through it.
                        pss = [psA.tile([128, 1024], F32, name="ps", tag="psa")
                               for _ in range(4)]
                        for kc in range(KC):
                            for sp in range(4):
                                for h2 in range(2):
                                    sl = sp * 2 + h2
                                    nc.tensor.matmul(
                                        pss[sp][:, h2 * 512:(h2 + 1) * 512],
                                        lhsT=xss[kc][:, sl * 128:(sl + 1) * 128],
                                        rhs=wv_sb[:, kc * GD:(kc + 1) * GD],
                                        start=(kc == 0),
                                        stop=(kc == KC - 1),
                                    )
                        issue_xs_dmas(1, 0)
                        issue_xs_dmas(1, 1)
                        for sp in range(4):
                            emit_v_evict(pss[sp], 0, sp)
                        continue
                    for m in range(4):
                        if m == 2 and gi + 1 < len(groups):
                            issue_xs_dmas(gi + 1, 0)
                        if m == 3 and gi + 1 < len(groups):
                            issue_xs_dmas(gi + 1, 1)
                        if xk == "v":
                            emit_v_chain(npair, m, xss)
                        elif xk == "k":
                            emit_qk_chain(wk_sb, kp_sb, kpT_out, npair, m, xss)
                        else:
                            emit_qk_chain(wq_sb, qp_sb, None, npair, m, xss)

            # ---------------- Phase B: attention + out-proj ----------------
            with (
                tc.tile_pool(name="mp", bufs=16) as mp,
                tc.tile_pool(name="apl", bufs=20) as apl,
                tc.tile_pool(name="cpl", bufs=8) as cpl,
                tc.tile_pool(name="stageB", bufs=12) as stageB,
                tc.tile_pool(name="ostp", bufs=6) as ostp,
                tc.tile_pool(name="psS", bufs=2, space="PSUM") as psS,
                tc.tile_pool(name="psCT", bufs=2, space="PSUM") as psCT,
                tc.tile_pool(name="psOT", bufs=2, space="PSUM") as psOT,
            ):
                steps = [(t, h) for t in range(NB) for h in range(GH)]
                mask_tiles = {}    # t -> list of 8 pair tiles
                at_tiles = {}      # (i, j) -> at pair tile
                cps_tiles = {}     # (i, jj//2) -> cps tile holding 2 chunks
                ctn_tiles = {}     # (i, jj) -> normalized ctx [128,128]
                cth_tiles = {}     # (t, h) -> [128,512] transposed ctx

                def emit_mask_dmas(t):
                    mts = []
                    for j in range(NJ):
                        mt = mp.tile([128, 1024], BF16, name="mt", tag="mt")
                        nc.sync.dma_start(
                            out=mt[:],
                            in_=maskP[j * 128:(j + 1) * 128,
                                      t * 1024:(t + 1) * 1024],
                        )
                        mts.append(mt)
                    mask_tiles[t] = mts

                def emit_scores_pair(i, j):
                    t, h = steps[i]
                    sps = psS.tile([128, 1024], F32, name="sps", tag="sps")
                    for c in range(2):
                        skc = 2 * j + c
                        nc.tensor.matmul(
                            sps[:, c * 512:(c + 1) * 512],
                            lhsT=kp_sb[:, h * S + skc * 128:
                                       h * S + (skc + 1) * 128],
                            rhs=qp_sb[:, h * S + t * 512: h * S + (t + 1) * 512],
                            start=True,
                            stop=True,
                        )
                    at = apl.tile([128, 1024], BF16, name="at", tag="at")
                    nc.scalar.activation(at[:], sps[:], AF.Exp, scale=SCALE)
                    nc.vector.tensor_mul(at[:], at[:], mask_tiles[t][j][:])
                    at_tiles[(i, j)] = at

                def emit_ctx_part(i, j):
                    # ctx matmuls for step i, slice j: mm-chunk jj = j//2,
                    # skc range (j%2)*8 .. +8; finalize (recip+norm) at odd j.
                    t, h = steps[i]
                    jj = j // 2
                    if j % 2 == 0 and jj % 2 == 0:
                        cps_tiles[(i, jj // 2)] = psCT.tile(
                            [128, 512], F32, name="cps", tag="cps")
                    cps = cps_tiles[(i, jj // 2)]
                    off = (jj % 2) * 129
                    for skc in range((j % 2) * 8, (j % 2) * 8 + 8):
                        at = at_tiles[(i, skc // 2)]
                        nc.tensor.matmul(
                            cps[:, off:off + 129],
                            lhsT=at[:, (skc % 2) * 512 + jj * 128:
                                    (skc % 2) * 512 + (jj + 1) * 128],
                            rhs=vpo_sb[:, skc * 516 + h * 129:
                                       skc * 516 + (h + 1) * 129],
                            start=(skc == 0),
                            stop=(skc == KC - 1),
                        )
                    if j % 2 == 1:
                        rec = stageB.tile([128, 1], F32, name="rec", tag="rec")
                        nc.vector.reciprocal(rec[:], cps[:, off + 128:off + 129])
                        ctn = stageB.tile([128, 128], BF16, name="ctn",
                                          tag="ctn")
                        nc.vector.tensor_scalar_mul(
                            ctn[:], cps[:, off:off + 128], rec[:])
                        ctn_tiles[(i, jj)] = ctn
                    if j == 7:
                        # release the at tiles of step i
                        for jd in range(NJ):
                            at_tiles.pop((i, jd), None)

                def emit_transposes(i):
                    t, h = steps[i]
                    tps = psOT.tile([128, 512], BF16, name="tps", tag="psot")
                    for mm in range(4):
                        nc.tensor.transpose(
                            tps[:, mm * 128:(mm + 1) * 128],
                            ctn_tiles.pop((i, mm))[:], ident[:])
                    cth = cpl.tile([128, 512], BF16, name="cth", tag="cth")
                    nc.vector.tensor_copy(cth[:], tps[:])
                    cth_tiles[(t, h)] = cth

                def emit_outproj(t):
                    cts = [cth_tiles.pop((t, h)) for h in range(GH)]
                    for mm in range(4):
                        for npair in range(2):
                            ops2 = [psOT.tile([128, 512], F32, name=f"ops{j2}",
                                              tag="psot") for j2 in range(2)]
                            for h in range(GH):
                                for n2 in range(2):
                                    n = npair * 2 + n2
                                    nc.tensor.matmul(
                                        ops2[n2][:],
                                        lhsT=cts[h][:, mm * 128:(mm + 1) * 128],
                                        rhs=wo_sb[:, h * D + n * 512:
                                                  h * D + (n + 1) * 512],
                                        start=(h == 0),
                                        stop=(h == GH - 1),
                                    )
                            ost = ostp.tile([128, 1024], BF16, name="ost",
                                            tag="ost")
                            nc.scalar.copy(ost[:, 0:512], ops2[0][:])
                            nc.vector.tensor_copy(ost[:, 512:1024], ops2[1][:])
                            eng = nc.sync if npair == 0 else nc.gpsimd
                            eng.dma_start(
                                out=outp_out[t * BLK + mm * 128:
                                             t * BLK + (mm + 1) * 128,
                                             npair * 1024:(npair + 1) * 1024],
                                in_=ost[:],
                            )

                # -------- software pipeline --------
                emit_mask_dmas(0)
                for i in range(len(steps) + 1):
                    t, h = steps[i] if i < len(steps) else (None, None)
                    for j in range(NJ):
                        if i < len(steps):
                            emit_scores_pair(i, j)
                        if j == 1 and i >= 2:
                            emit_transposes(i - 2)
                        if j == 3 and i < len(steps) and h == 2:
                            emit_mask_dmas(t + 1) if t + 1 < NB else None
                        if i >= 1:
                            emit_ctx_part(i - 1, j)
                    if i >= 5 and steps[i - 1][1] == 0:
                        # transposes of block t'-1 all done (last at i-2, j=1)
                        emit_outproj(steps[i - 2][0])
                # epilogue: transposes of the last two steps, final outproj
                emit_transposes(len(steps) - 1)
                emit_outproj(NB - 1)

    nc.compile()
    return nc


def get_nc():
    if "nc" not in _CACHE:
        _CACHE["nc"] = _build()
    return _CACHE["nc"]


def make_in_maps(inputs):
    q = np.asarray(inputs["q"], np.float32)
    k = np.asarray(inputs["k"], np.float32)
    v = np.asarray(inputs["v"], np.float32)
    mask = np.asarray(inputs["mask"])
    Wq = np.asarray(inputs["Wq"], np.float32)
    Wk = np.asarray(inputs["Wk"], np.float32)
    Wv = np.asarray(inputs["Wv"], np.float32)
    Wo = np.asarray(inputs["Wo"], np.float32)

    per_batch = []
    for b in range(B):
        maskTb = np.ascontiguousarray(
            (~mask[b].astype(bool)).T).astype(nbf16)  # [key, q]
        # pair-packed mask: row j*128+p, col t*1024 + c*512 + g
        #   = maskTb[j*256 + c*128 + p, t*512 + g]
        maskP = np.ascontiguousarray(
            maskTb.reshape(NJ, 2, 128, NB, 512)
            .transpose(0, 2, 3, 1, 4).reshape(S // 2, 2 * S))
        per_batch.append({
            "qT": np.ascontiguousarray(q[b].T).astype(nbf16),
            "kT": np.ascontiguousarray(k[b].T).astype(nbf16),
            "vT": np.ascontiguousarray(v[b].T).astype(nbf16),
            "maskP": maskP,
        })

    def packw(wT, ncols):
        # wT [rows, ncols] -> [128, (rows//128)*ncols] with chunk-major cols
        r = wT.shape[0] // 128
        return np.ascontiguousarray(
            wT.reshape(r, 128, ncols).transpose(1, 0, 2).reshape(128, r * ncols))

    per_group = []
    for g in range(4):
        sl = slice(g * GD, (g + 1) * GD)
        per_group.append({
            "wqP": packw(np.ascontiguousarray(Wq[sl, :].T).astype(nbf16), GD),
            "wkP": packw(np.ascontiguousarray(Wk[sl, :].T).astype(nbf16), GD),
            "wvP": packw(np.ascontiguousarray(Wv[sl, :].T).astype(nbf16), GD),
            "woP": packw(np.ascontiguousarray(Wo[:, sl].T).astype(nbf16), D),
        })
    in_maps = []
    for c in range(N_CORES):
        b, g = c // 4, c % 4
        m = {}
        m.update(per_batch[b])
        m.update(per_group[g])
        in_maps.append(m)
    return in_maps


def assemble(results):
    out = np.zeros((B, S, D), np.float32)
    kp = np.empty((B, S, D), np.float32)
    vp = np.empty((B, S, D), np.float32)
    for c, res in enumerate(results):
        b, g = c // 4, c % 4
        kp[b][:, g * GD:(g + 1) * GD] = res["kpT_out"].T
        vp[b][:, g * GD:(g + 1) * GD] = res["vp_out"]
        out[b] += res["outp_out"].astype(np.float32)
    return out, kp, vp


def run_cores(in_maps, trace=False, **kwargs):
    nc = get_nc()
    return bass_utils.run_bass_kernel_spmd(
        nc, in_maps, core_ids=list(range(N_CORES)), trace=trace, **kwargs
    )


def kernel(**inputs):
    in_maps = make_in_maps(inputs)
    res = run_cores(in_maps, trace=False)
    return assemble(res.results)
